# revision 5
# baseline (speedup 1.0000x reference)
"""GCN encoder (3-layer, PyG GCNConv-style) on 8 Trainium2 NeuronCores.

Strategy (per spec sharding hint):
  - 1D node partition: core c owns rows [c*8192, (c+1)*8192).
  - Per layer: local transform (h * deg^-1/2) @ W -> bf16 "table" slice,
    AllGather -> full [65536, 128] bf16 table in DRAM on every core.
  - Edges partitioned by destination owner (host preprocessing). Per
    128-dst block, source rows are fetched with dma_gather (int16 indices
    -> two 32768-row windows, rebased) and aggregated with
    msgs^T @ onehot(dst_local) matmuls accumulating in PSUM. Pad slots use
    idx=0 with dst_local=-1 (zero one-hot column -> no contribution).
  - deg^-1/2 is folded into the table rows (source side) and applied once
    per destination after aggregation; the self-loop term is the core's own
    (already scaled) slice added via an identity matmul.
  - Layer 3 aggregates h2 first (width 128) and applies W3 after.
"""
import os
import numpy as np
import ml_dtypes

import concourse.bass as bass
import concourse.tile as tile
from concourse import bacc, mybir
from concourse import bass_utils

N, E, IN, H, Z = 65536, 524288, 256, 128, 32
LN_EPS = 1e-5
NCORES = 8

F32 = mybir.dt.float32
BF16 = mybir.dt.bfloat16
I16 = mybir.dt.int16


def _ceil_div(a, b):
    return -(-a // b)


def _preprocess(edge_index, n=N):
    """Host-side index preprocessing. Returns per-core gather structures."""
    ns = n // NCORES
    ntn = ns // 128           # dst blocks per core
    half = n // 2
    src = np.asarray(edge_index[0], dtype=np.int64)
    dst = np.asarray(edge_index[1], dtype=np.int64)
    deg = (np.bincount(dst, minlength=n) + 1.0).astype(np.float32)

    per_core = []
    counts = np.zeros((NCORES, 2, ntn), np.int64)
    for c in range(NCORES):
        m = (dst >= c * ns) & (dst < (c + 1) * ns)
        s_c = src[m]
        d_c = dst[m] - c * ns
        w_c = (s_c >= half).astype(np.int64)
        b_c = d_c >> 7
        order = np.lexsort((d_c, b_c, w_c))
        s_c, d_c, w_c, b_c = s_c[order], d_c[order], w_c[order], b_c[order]
        counts[c] = np.bincount(w_c * ntn + b_c,
                                minlength=2 * ntn).reshape(2, ntn)
        per_core.append((s_c, d_c, w_c))
    # tiles (of 128 slots) per (window, block): uniform across cores
    K = _ceil_div(counts.max(axis=0), 128).astype(np.int64)  # [2, ntn]
    SL = (K.sum(axis=1) * 128).astype(np.int64)              # slots per window
    offs = np.concatenate([np.zeros((2, 1), np.int64),
                           np.cumsum(K * 128, axis=1)], axis=1)  # slot offsets

    cores = []
    for c in range(NCORES):
        s_c, d_c, w_c = per_core[c]
        idx_arrs, dstl_arrs = [], []
        for w in range(2):
            idx = np.zeros(SL[w], np.int16)
            dstl = np.full(SL[w], -1.0, np.float32)
            sw = s_c[w_c == w] - w * half
            dw = d_c[w_c == w]
            bw = dw >> 7
            # counts per block for this core/window
            cnt = np.bincount(bw, minlength=ntn)
            pos = offs[w][bw] + (np.arange(len(sw)) -
                                 np.concatenate([[0], np.cumsum(cnt)])[bw])
            idx[pos] = sw.astype(np.int16)
            dstl[pos] = (dw & 127).astype(np.float32)
            # wrap idx: slot j -> partition j%16, col j//16; replicate x8
            idx_w = np.tile(idx.reshape(-1, 16).T, (8, 1)).astype(np.int16)
            dstl_w = np.ascontiguousarray(
                dstl.reshape(-1, 128).T.astype(np.float32))
            idx_arrs.append(np.ascontiguousarray(idx_w))
            dstl_arrs.append(dstl_w)
        cores.append((idx_arrs, dstl_arrs))
    return deg, K, SL, cores


def _build_program(K, SL, n=N):
    ns = n // NCORES
    ntn = ns // 128
    half = n // 2
    nc = bacc.Bacc("TRN2", target_bir_lowering=False, debug=False,
                   enable_asserts=False, num_devices=NCORES)

    dt = nc.dram_tensor
    x_t = dt("x_sh", [ns, IN], F32, kind="ExternalInput").ap()
    degb_t = dt("deg_bc", [128, ns], F32, kind="ExternalInput").ap()
    degnm_t = dt("deg_nm", [128, ntn], F32, kind="ExternalInput").ap()
    w1_t = dt("W1", [IN, H], F32, kind="ExternalInput").ap()
    w2_t = dt("W2", [H, H], F32, kind="ExternalInput").ap()
    w3_t = dt("W3", [H, Z], F32, kind="ExternalInput").ap()
    b1_t = dt("b1", [128, 1], F32, kind="ExternalInput").ap()
    b2_t = dt("b2", [128, 1], F32, kind="ExternalInput").ap()
    b3_t = dt("b3_bc", [128, Z], F32, kind="ExternalInput").ap()
    lnw_t = dt("lnw_bc", [128, IN], F32, kind="ExternalInput").ap()
    lnb_t = dt("lnb_bc", [128, IN], F32, kind="ExternalInput").ap()
    idf_t = dt("id_f32", [128, 128], F32, kind="ExternalInput").ap()
    idb_t = dt("id_bf16", [128, 128], BF16, kind="ExternalInput").ap()
    iota_t = dt("iota", [128, 128], F32, kind="ExternalInput").ap()
    idx_t = [dt(f"idx{w}", [128, SL[w] // 16], I16, kind="ExternalInput").ap()
             for w in range(2)]
    dsl_t = [dt(f"dstl{w}", [128, SL[w] // 128], F32,
                kind="ExternalInput").ap() for w in range(2)]
    out_t = dt("out", [ns, Z], F32, kind="ExternalOutput").ap()

    koffs = np.concatenate([np.zeros((2, 1), np.int64),
                            np.cumsum(K, axis=1)], axis=1)  # tile offsets

    with tile.TileContext(nc) as tc:
        with tc.tile_pool(name="consts", bufs=1) as consts, \
             tc.tile_pool(name="state", bufs=1) as state, \
             tc.tile_pool(name="work", bufs=4) as work, \
             tc.tile_pool(name="small", bufs=4) as small, \
             tc.tile_pool(name="msgsp", bufs=4) as msgsp, \
             tc.tile_pool(name="psum_tp", bufs=2, space="PSUM") as psum_tp, \
             tc.tile_pool(name="psum_tf", bufs=2, space="PSUM") as psum_tf, \
             tc.tile_pool(name="psum_ag", bufs=2, space="PSUM") as psum_ag, \
             tc.tile_pool(name="dram", bufs=1, space="DRAM") as dram:

            cc_in = [dram.tile([ns, H], BF16, tag=f"cc{i}", name=f"cc{i}")
                     for i in range(3)]
            tables = [dram.tile([n, H], BF16, tag=f"tab{i}", name=f"tab{i}",
                                addr_space="Shared")
                      for i in range(3)]

            # ---- load constants ----
            def cload(ap_in, shape, dtype, tag):
                t = consts.tile(shape, dtype, tag=tag, name=tag)
                nc.sync.dma_start(t[:], ap_in)
                return t

            w1_sb = cload(w1_t.rearrange("(k p) f -> p k f", p=128),
                          [128, 2, H], F32, "w1")
            w2_sb = cload(w2_t[:], [128, H], F32, "w2")
            w3_sb = cload(w3_t[:], [128, Z], F32, "w3")
            b1_sb = cload(b1_t[:], [128, 1], F32, "b1")
            b2_sb = cload(b2_t[:], [128, 1], F32, "b2")
            b3_sb = cload(b3_t[:], [128, Z], F32, "b3")
            lnw_sb = cload(lnw_t[:], [128, IN], F32, "lnw")
            lnb_sb = cload(lnb_t[:], [128, IN], F32, "lnb")
            idf_sb = cload(idf_t[:], [128, 128], F32, "idf")
            idb_sb = cload(idb_t[:], [128, 128], BF16, "idb")
            iota_sb = cload(iota_t[:], [128, 128], F32, "iota")
            idx_sb = [cload(idx_t[w][:], [128, SL[w] // 16], I16, f"idx{w}")
                      for w in range(2)]
            dsl_sb = [cload(dsl_t[w][:], [128, SL[w] // 128], F32, f"dsl{w}")
                      for w in range(2)]

            # dis = 1/sqrt(deg), broadcast [128, ns] and node-major [128, ntn]
            degb_sb = consts.tile([128, ns], F32, tag="degb")
            nc.sync.dma_start(degb_sb[:], degb_t[:])
            disb = consts.tile([128, ns], F32, tag="disb")
            nc.scalar.activation(disb[:], degb_sb[:],
                                 mybir.ActivationFunctionType.Sqrt)
            nc.vector.reciprocal(disb[:], disb[:])
            degnm_sb = consts.tile([128, ntn], F32, tag="degnm")
            nc.sync.dma_start(degnm_sb[:], degnm_t[:])
            disnm = consts.tile([128, ntn], F32, tag="disnm")
            nc.scalar.activation(disnm[:], degnm_sb[:],
                                 mybir.ActivationFunctionType.Sqrt)
            nc.vector.reciprocal(disnm[:], disnm[:])

            slice_sb = state.tile([128, ntn * H], BF16, tag="slice")
            hA = state.tile([128, ns], F32, tag="hA")   # h1T_s then agg3T_s
            hB = state.tile([128, ns], F32, tag="hB")   # h2T_s

            # ---- Layer 1 transform: LN + scale + @W1 ----
            for t in range(ntn):
                xt = work.tile([128, IN], F32, tag="xt")
                nc.sync.dma_start(xt[:], x_t[t * 128:(t + 1) * 128, :])
                s1 = small.tile([128, 1], F32, tag="s1")
                nc.vector.reduce_sum(s1[:], xt[:], axis=mybir.AxisListType.X)
                negmu = small.tile([128, 1], F32, tag="negmu")
                nc.vector.tensor_scalar(out=negmu[:], in0=s1[:],
                                        scalar1=-1.0 / IN, scalar2=None,
                                        op0=mybir.AluOpType.mult)
                sq = work.tile([128, IN], F32, tag="sq")
                s2 = small.tile([128, 1], F32, tag="s2")
                nc.scalar.activation(sq[:], xt[:],
                                     mybir.ActivationFunctionType.Square,
                                     bias=negmu[:, 0:1], accum_out=s2[:])
                veps = small.tile([128, 1], F32, tag="veps")
                nc.vector.tensor_scalar(out=veps[:], in0=s2[:],
                                        scalar1=1.0 / IN, scalar2=LN_EPS,
                                        op0=mybir.AluOpType.mult,
                                        op1=mybir.AluOpType.add)
                rec = small.tile([128, 1], F32, tag="rec")
                nc.vector.reciprocal(rec[:], veps[:])
                rstd = small.tile([128, 1], F32, tag="rstd")
                nc.scalar.activation(rstd[:], rec[:],
                                     mybir.ActivationFunctionType.Sqrt)
                h = work.tile([128, IN], F32, tag="h")
                nc.vector.tensor_scalar(out=h[:], in0=xt[:],
                                        scalar1=negmu[:, 0:1],
                                        scalar2=rstd[:, 0:1],
                                        op0=mybir.AluOpType.add,
                                        op1=mybir.AluOpType.mult)
                nc.vector.tensor_tensor(out=h[:], in0=h[:], in1=lnw_sb[:],
                                        op=mybir.AluOpType.mult)
                nc.vector.tensor_tensor(out=h[:], in0=h[:], in1=lnb_sb[:],
                                        op=mybir.AluOpType.add)
                nc.vector.tensor_scalar(out=h[:], in0=h[:],
                                        scalar1=disnm[:, t:t + 1],
                                        scalar2=None,
                                        op0=mybir.AluOpType.mult)
                tf = psum_tf.tile([128, H], F32, tag="tf")
                for k in range(2):
                    tp = psum_tp.tile([128, 128], F32, tag="tp")
                    nc.tensor.transpose(tp[:], h[:, k * 128:(k + 1) * 128],
                                        idf_sb[:])
                    xT = work.tile([128, 128], F32, tag="xT")
                    nc.vector.tensor_copy(xT[:], tp[:])
                    nc.tensor.matmul(out=tf[:], lhsT=xT[:],
                                     rhs=w1_sb[:, k, :],
                                     start=(k == 0), stop=(k == 1))
                nc.vector.tensor_copy(slice_sb[:, t * H:(t + 1) * H], tf[:])

            nc.sync.dma_start(
                cc_in[0][:].rearrange("(t p) f -> p t f", p=128), slice_sb[:])
            nc.gpsimd.collective_compute(
                "AllGather", mybir.AluOpType.bypass,
                replica_groups=[list(range(NCORES))],
                ins=[cc_in[0][:].opt()], outs=[tables[0][:].opt()])

            # ---- aggregation helper ----
            def agg(table, layer):
                """Accumulate per dst-block into psum; call epilogue."""
                for b in range(ntn):
                    pa = psum_ag.tile([128, 128], F32, tag="pa")
                    nmm = int(K[0][b] + K[1][b])
                    if layer < 3:
                        nc.tensor.matmul(out=pa[:],
                                         lhsT=slice_sb[:, b * H:(b + 1) * H],
                                         rhs=idb_sb[:],
                                         start=True, stop=(nmm == 0))
                        started = True
                    else:
                        started = False
                    done = 0
                    for w in range(2):
                        Kb = int(K[w][b])
                        if Kb == 0:
                            continue
                        mg = msgsp.tile([128, Kb, H], BF16, tag="mg")
                        nc.gpsimd.dma_gather(
                            out_ap=mg[:],
                            in_ap=table[w * half:(w + 1) * half, :],
                            idxs_ap=idx_sb[w][:, koffs[w][b] * 8:
                                              (koffs[w][b] + Kb) * 8],
                            num_idxs=Kb * 128, num_idxs_reg=Kb * 128,
                            elem_size=H)
                        for j in range(Kb):
                            col = koffs[w][b] + j
                            oh = work.tile([128, 128], BF16, tag="oh")
                            nc.vector.tensor_scalar(
                                out=oh[:], in0=iota_sb[:],
                                scalar1=dsl_sb[w][:, col:col + 1],
                                scalar2=None,
                                op0=mybir.AluOpType.is_equal)
                            done += 1
                            nc.tensor.matmul(
                                out=pa[:], lhsT=mg[:, j, :], rhs=oh[:],
                                start=not started, stop=(done == nmm))
                            started = True
                    yield b, pa, (nmm > 0 or layer < 3)

            # ---- Layer 1 aggregate -> h1T_s (scaled) in hA ----
            for b, pa, _ in agg(tables[0], 1):
                cols = slice(b * 128, (b + 1) * 128)
                t1 = work.tile([128, 128], F32, tag="ep1")
                nc.vector.tensor_tensor(out=t1[:], in0=pa[:],
                                        in1=disb[:, cols],
                                        op=mybir.AluOpType.mult)
                t2 = work.tile([128, 128], F32, tag="ep2")
                nc.scalar.activation(t2[:], t1[:],
                                     mybir.ActivationFunctionType.Relu,
                                     bias=b1_sb[:, 0:1])
                nc.vector.tensor_tensor(out=hA[:, cols], in0=t2[:],
                                        in1=disb[:, cols],
                                        op=mybir.AluOpType.mult)

            # ---- Layer 2 transform + AllGather ----
            for t in range(ntn):
                tf = psum_tf.tile([128, H], F32, tag="tf")
                nc.tensor.matmul(out=tf[:],
                                 lhsT=hA[:, t * 128:(t + 1) * 128],
                                 rhs=w2_sb[:], start=True, stop=True)
                nc.vector.tensor_copy(slice_sb[:, t * H:(t + 1) * H], tf[:])
            nc.sync.dma_start(
                cc_in[1][:].rearrange("(t p) f -> p t f", p=128), slice_sb[:])
            nc.gpsimd.collective_compute(
                "AllGather", mybir.AluOpType.bypass,
                replica_groups=[list(range(NCORES))],
                ins=[cc_in[1][:].opt()], outs=[tables[1][:].opt()])

            # ---- Layer 2 aggregate -> h2T_s (scaled) in hB ----
            for b, pa, _ in agg(tables[1], 2):
                cols = slice(b * 128, (b + 1) * 128)
                t1 = work.tile([128, 128], F32, tag="ep1")
                nc.vector.tensor_tensor(out=t1[:], in0=pa[:],
                                        in1=disb[:, cols],
                                        op=mybir.AluOpType.mult)
                t2 = work.tile([128, 128], F32, tag="ep2")
                nc.scalar.activation(t2[:], t1[:],
                                     mybir.ActivationFunctionType.Relu,
                                     bias=b2_sb[:, 0:1])
                nc.vector.tensor_tensor(out=hB[:, cols], in0=t2[:],
                                        in1=disb[:, cols],
                                        op=mybir.AluOpType.mult)

            # ---- Layer 3: table = h2T_s transposed to node-major ----
            for t in range(ntn):
                tp = psum_tp.tile([128, 128], F32, tag="tp")
                nc.tensor.transpose(tp[:], hB[:, t * 128:(t + 1) * 128],
                                    idf_sb[:])
                nc.vector.tensor_copy(slice_sb[:, t * H:(t + 1) * H], tp[:])
            nc.sync.dma_start(
                cc_in[2][:].rearrange("(t p) f -> p t f", p=128), slice_sb[:])
            nc.gpsimd.collective_compute(
                "AllGather", mybir.AluOpType.bypass,
                replica_groups=[list(range(NCORES))],
                ins=[cc_in[2][:].opt()], outs=[tables[2][:].opt()])

            # ---- Layer 3 aggregate (no relu/bias; self via DVE add) ----
            out_sb = state.tile([128, ntn * Z], F32, tag="outsb")
            for b, pa, has_mm in agg(tables[2], 3):
                cols = slice(b * 128, (b + 1) * 128)
                t1 = work.tile([128, 128], F32, tag="ep1")
                if has_mm:
                    nc.vector.tensor_tensor(out=t1[:], in0=pa[:],
                                            in1=hB[:, cols],
                                            op=mybir.AluOpType.add)
                else:
                    nc.vector.tensor_copy(t1[:], hB[:, cols])
                nc.vector.tensor_tensor(out=hA[:, cols], in0=t1[:],
                                        in1=disb[:, cols],
                                        op=mybir.AluOpType.mult)
                # final transform for this block
                po = psum_tf.tile([128, Z], F32, tag="tf")
                nc.tensor.matmul(out=po[:], lhsT=hA[:, cols], rhs=w3_sb[:],
                                 start=True, stop=True)
                nc.vector.tensor_tensor(out=out_sb[:, b * Z:(b + 1) * Z],
                                        in0=po[:], in1=b3_sb[:],
                                        op=mybir.AluOpType.add)

            nc.sync.dma_start(
                out_t.rearrange("(t p) z -> p t z", p=128), out_sb[:])

    nc.compile()
    return nc


def _make_in_maps(x, edge_index, ln_w, ln_b, W1, b1, W2, b2, W3, b3, n=N):
    ns = n // NCORES
    ntn = ns // 128
    deg, K, SL, cores = _preprocess(edge_index, n)

    x = np.asarray(x, np.float32)
    iota_np = np.broadcast_to(np.arange(128, dtype=np.float32),
                              (128, 128)).copy()
    id_f32 = np.eye(128, dtype=np.float32)
    id_bf16 = np.eye(128, dtype=ml_dtypes.bfloat16)
    shared = dict(
        W1=np.asarray(W1, np.float32), W2=np.asarray(W2, np.float32),
        W3=np.asarray(W3, np.float32),
        b1=np.asarray(b1, np.float32).reshape(128, 1),
        b2=np.asarray(b2, np.float32).reshape(128, 1),
        b3_bc=np.broadcast_to(np.asarray(b3, np.float32), (128, Z)).copy(),
        lnw_bc=np.broadcast_to(np.asarray(ln_w, np.float32), (128, IN)).copy(),
        lnb_bc=np.broadcast_to(np.asarray(ln_b, np.float32), (128, IN)).copy(),
        id_f32=id_f32, id_bf16=id_bf16, iota=iota_np,
    )
    in_maps = []
    for c in range(NCORES):
        idx_arrs, dstl_arrs = cores[c]
        dg = deg[c * ns:(c + 1) * ns]
        m = dict(shared)
        m["x_sh"] = np.ascontiguousarray(x[c * ns:(c + 1) * ns])
        m["deg_bc"] = np.broadcast_to(dg, (128, ns)).copy()
        m["deg_nm"] = np.ascontiguousarray(dg.reshape(ntn, 128).T)
        for w in range(2):
            m[f"idx{w}"] = idx_arrs[w]
            m[f"dstl{w}"] = dstl_arrs[w]
        in_maps.append(m)
    return K, SL, in_maps


def kernel(x, edge_index, ln_w, ln_b, W1, b1, W2, b2, W3, b3):
    K, SL, in_maps = _make_in_maps(x, edge_index, ln_w, ln_b,
                                   W1, b1, W2, b2, W3, b3)
    nc = _build_program(K, SL)
    res = bass_utils.run_bass_kernel_spmd(
        nc, in_maps, core_ids=list(range(NCORES)), trace=False)
    out = np.concatenate([res.results[c]["out"] for c in range(NCORES)],
                         axis=0)
    return out.astype(np.float32)
